# revision 1
# baseline (speedup 1.0000x reference)
"""Longformer sliding-window self-attention (MBart variant) on 8 TRN2 cores.

Strategy: sequence-parallel sharding. Each of the 8 cores gets one
(batch, quarter-sequence) shard: core c -> batch c//4, queries
[1024*(c%4), 1024*(c%4+1)). Each core receives a halo'd slice of the
hidden states (1536 rows, transposed, zero-padded at sequence edges),
computes Q/K/V projections, banded attention over 4 chunks of 256
queries x 768-key windows, and the output projection, returning its
[768, 1024] transposed output slice. Host re-assembles the full
[2, 4096, 768] output.

All matmuls run as float32r (full PE rate, ~1e-4 component error).

Math notes (exact rewrites of the reference):
  - query scale 1/sqrt(64) folded into Wq/bq on host.
  - Wk bias drops out of softmax entirely (constant per query row).
  - Wv bias commutes through softmax (weights sum to 1) and the output
    projection: folded into bo_eff = bo + Wo @ bv on host.
  - band + sequence-edge + attention_mask key bias folded into additive
    per-core mask tiles built on host (NEG = -1e9 outside the band).
  - softmax computed unnormalized; the denominator is produced by an
    extra all-ones column appended to each head's V block, and applied
    as a reciprocal multiply on the context rows.
"""

import numpy as np

# problem shapes (fixed by the task)
B, S, D, H = 2, 4096, 768, 12
DH = D // H            # 64
W = 256                # one-sided window == chunk size b
NEG = -1e9
NCORES = 8
G = 4                  # sequence groups per batch (8 cores / 2 batches)
SLOC = S // G          # 1024 queries per core
SH = SLOC + 2 * W      # 1536 halo'd rows per core
NB = SLOC // W         # 4 chunks per core
NT = 3 * W // 128      # 6 key tiles of 128 per chunk window
P = 128
DJ = D // P            # 6 tiles of 128 over the model dim

_PROGRAM_CACHE: dict = {}


def _build_program(general_mask: bool):
    import concourse.bacc as bacc
    import concourse.mybir as mybir
    import concourse.tile as tile
    from contextlib import ExitStack

    F32 = mybir.dt.float32
    F32R = mybir.dt.float32r
    AF = mybir.ActivationFunctionType
    NS = 3 if general_mask else 2        # mask slots per chunk
    MCOLS = NB * NS * 512                # mask sbuf columns

    nc = bacc.Bacc("TRN2", target_bir_lowering=False, debug=False)

    hsT = nc.dram_tensor("hsT", [D, SH], F32R, kind="ExternalInput")
    wqT = nc.dram_tensor("wqT", [D, D], F32R, kind="ExternalInput")
    wkT = nc.dram_tensor("wkT", [D, D], F32R, kind="ExternalInput")
    wvT = nc.dram_tensor("wvT", [D, D], F32R, kind="ExternalInput")
    woT = nc.dram_tensor("woT", [D, D], F32R, kind="ExternalInput")
    bq = nc.dram_tensor("bq", [D], F32, kind="ExternalInput")
    boe = nc.dram_tensor("boe", [D], F32, kind="ExternalInput")
    masks = nc.dram_tensor("masks", [NB, NS, P, 512], F32, kind="ExternalInput")
    outT = nc.dram_tensor("outT", [D, SLOC], F32, kind="ExternalOutput")

    with tile.TileContext(nc) as tc, ExitStack() as stack:
        const = stack.enter_context(tc.tile_pool(name="const", bufs=1))
        qt_p = stack.enter_context(tc.tile_pool(name="qt", bufs=1))
        kt_p = stack.enter_context(tc.tile_pool(name="kt", bufs=1))
        ct_p = stack.enter_context(tc.tile_pool(name="ct", bufs=1))

        bq_sb = const.tile([P, DJ], F32, tag="bq")
        nc.sync.dma_start(out=bq_sb[:], in_=bq.rearrange("(t p) -> p t", p=P))
        boe_sb = const.tile([P, DJ], F32, tag="boe")
        nc.sync.dma_start(out=boe_sb[:], in_=boe.rearrange("(t p) -> p t", p=P))
        mask_sb = const.tile([P, MCOLS], F32, tag="masks")
        for n in range(NB):
            for sl in range(NS):
                off = (n * NS + sl) * 512
                nc.sync.dma_start(
                    out=mask_sb[:, off : off + 512], in_=masks[n, sl]
                )

        QT = [qt_p.tile([P, SLOC], F32R, tag=f"qt{j}", name=f"qt{j}") for j in range(DJ)]
        KT = [kt_p.tile([P, SH], F32R, tag=f"kt{j}", name=f"kt{j}") for j in range(DJ)]
        CT = [ct_p.tile([P, SLOC], F32R, tag=f"ct{j}", name=f"ct{j}") for j in range(DJ)]

        # ---------------- phase 1: projections ------------------------
        # pool stack is LIFO: va (persistent) must open before hs.
        va_p = stack.enter_context(tc.tile_pool(name="va", bufs=1))
        VA = [va_p.tile([P, H * (DH + 1)], F32R, tag=f"va{s}", name=f"va{s}") for s in range(SH // P)]

        ones_f = const.tile([P, H], F32, tag="ones_f")
        nc.vector.memset(ones_f[:], 1.0)
        for st in range(SH // P):
            view = VA[st].rearrange("p (h e) -> p h e", e=DH + 1)
            nc.vector.tensor_copy(view[:, :, DH : DH + 1], ones_f[:])

        hs_stack = ExitStack()
        hs_p = hs_stack.enter_context(tc.tile_pool(name="hs", bufs=1))
        HS = [hs_p.tile([P, SH], F32R, tag=f"hs{i}", name=f"hs{i}") for i in range(DJ)]
        for i in range(DJ):
            nc.sync.dma_start(out=HS[i][:], in_=hsT[P * i : P * (i + 1), :])

        # V = hs @ Wv.T  -> VA[st] tiles [128 seq, 768 head-dims]
        with (
            tc.tile_pool(name="wv", bufs=1) as wv_p,
            tc.tile_pool(name="ps2", bufs=4, space="PSUM") as ps2,
        ):
            for half in range(2):
                WV = [wv_p.tile([P, 384], F32R, tag=f"wv{i}", name=f"wv{i}") for i in range(DJ)]
                for i in range(DJ):
                    nc.sync.dma_start(
                        out=WV[i][:],
                        in_=wvT[P * i : P * (i + 1), 384 * half : 384 * (half + 1)],
                    )
                for st in range(SH // P):
                    ps = ps2.tile([P, 384], F32, tag="ps2")
                    for i in range(DJ):
                        nc.tensor.matmul(
                            ps[:],
                            HS[i][:, P * st : P * (st + 1)],
                            WV[i][:],
                            start=(i == 0),
                            stop=(i == DJ - 1),
                        )
                    view = VA[st].rearrange("p (h e) -> p h e", e=DH + 1)
                    nc.vector.tensor_copy(
                        view[:, 6 * half : 6 * (half + 1), 0:DH],
                        ps[:].rearrange("p (h e) -> p h e", e=DH),
                    )

        # QT[j] = (Wq/8) @ hs_loc.T + bq/8
        with (
            tc.tile_pool(name="wq", bufs=1) as wq_p,
            tc.tile_pool(name="ps1", bufs=4, space="PSUM") as ps1,
        ):
            for half in range(2):
                WQ = [wq_p.tile([P, 384], F32R, tag=f"wq{i}", name=f"wq{i}") for i in range(DJ)]
                for i in range(DJ):
                    nc.sync.dma_start(
                        out=WQ[i][:],
                        in_=wqT[P * i : P * (i + 1), 384 * half : 384 * (half + 1)],
                    )
                for j in range(3 * half, 3 * half + 3):
                    jc = P * j - 384 * half
                    for sp in range(SLOC // 512):
                        ps = ps1.tile([P, 512], F32, tag="ps1")
                        for i in range(DJ):
                            nc.tensor.matmul(
                                ps[:],
                                WQ[i][:, jc : jc + P],
                                HS[i][:, W + 512 * sp : W + 512 * (sp + 1)],
                                start=(i == 0),
                                stop=(i == DJ - 1),
                            )
                        nc.scalar.activation(
                            QT[j][:, 512 * sp : 512 * (sp + 1)],
                            ps[:],
                            AF.Identity,
                            bias=bq_sb[:, j : j + 1],
                        )

        # KT[j] = Wk @ hs_halo.T  (bias bk cancels in softmax)
        with (
            tc.tile_pool(name="wk", bufs=1) as wk_p,
            tc.tile_pool(name="ps1b", bufs=4, space="PSUM") as ps1b,
        ):
            for half in range(2):
                WK = [wk_p.tile([P, 384], F32R, tag=f"wk{i}", name=f"wk{i}") for i in range(DJ)]
                for i in range(DJ):
                    nc.sync.dma_start(
                        out=WK[i][:],
                        in_=wkT[P * i : P * (i + 1), 384 * half : 384 * (half + 1)],
                    )
                for j in range(3 * half, 3 * half + 3):
                    jc = P * j - 384 * half
                    for sp in range(SH // 512):
                        ps = ps1b.tile([P, 512], F32, tag="ps1b")
                        for i in range(DJ):
                            nc.tensor.matmul(
                                ps[:],
                                WK[i][:, jc : jc + P],
                                HS[i][:, 512 * sp : 512 * (sp + 1)],
                                start=(i == 0),
                                stop=(i == DJ - 1),
                            )
                        nc.scalar.activation(
                            KT[j][:, 512 * sp : 512 * (sp + 1)], ps[:], AF.Copy
                        )
        hs_stack.close()

        # ---------------- phase 2: banded attention -------------------
        with (
            tc.tile_pool(name="expp", bufs=3) as exp_p,
            tc.tile_pool(name="dn", bufs=4) as dn_p,
            tc.tile_pool(name="pss", bufs=2, space="PSUM") as pss,
            tc.tile_pool(name="psc", bufs=2, space="PSUM") as psc,
        ):
            for n in range(NB):
                for j in range(DJ):
                    sps = [pss.tile([P, NT * W], F32, tag="s", name=f"s{n}_{j}_{k}") for k in range(2)]
                    for t in range(NT):
                        for hh in range(2):
                            r0 = DH * hh
                            nc.tensor.matmul(
                                sps[hh][:, W * t : W * (t + 1)],
                                KT[j][r0 : r0 + DH, W * n + P * t : W * n + P * (t + 1)],
                                QT[j][r0 : r0 + DH, W * n : W * (n + 1)],
                                start=True,
                                stop=True,
                            )
                    for hh in range(2):
                        h = 2 * j + hh
                        moff = n * NS * 512
                        nc.vector.tensor_add(
                            sps[hh][:, 0:512],
                            sps[hh][:, 0:512],
                            mask_sb[:, moff : moff + 512],
                        )
                        if general_mask:
                            nc.vector.tensor_add(
                                sps[hh][:, 512:1024],
                                sps[hh][:, 512:1024],
                                mask_sb[:, moff + 512 : moff + 1024],
                            )
                        lastoff = moff + (NS - 1) * 512
                        nc.vector.tensor_add(
                            sps[hh][:, 1024:1536],
                            sps[hh][:, 1024:1536],
                            mask_sb[:, lastoff : lastoff + 512],
                        )
                        expt = exp_p.tile([P, NT * W], F32R, tag="e", name=f"e{n}_{j}_{hh}")
                        nc.scalar.activation(expt[:], sps[hh][:], AF.Exp)
                        cps = psc.tile([DH + 1, W], F32, tag="c", name=f"c{n}_{h}")
                        for t in range(NT):
                            nc.tensor.matmul(
                                cps[:],
                                VA[2 * n + t][:, (DH + 1) * h : (DH + 1) * (h + 1)],
                                expt[:, W * t : W * (t + 1)],
                                start=(t == 0),
                                stop=(t == NT - 1),
                            )
                        # denominator row -> partition 0 (DMA hop), reciprocal,
                        # broadcast over the head's 64 lanes, fused normalize
                        dcp = dn_p.tile([DH + 1, W], F32, tag="dcp", name=f"dcp{n}_{h}")
                        nc.vector.tensor_copy(dcp[DH : DH + 1, :], cps[DH : DH + 1, :])
                        dnrow = dn_p.tile([1, W], F32, tag="dnr", name=f"dnr{n}_{h}")
                        nc.sync.dma_start(out=dnrow[:], in_=dcp[DH : DH + 1, :])
                        rcrow = dn_p.tile([1, W], F32, tag="rcr", name=f"rcr{n}_{h}")
                        scr = dn_p.tile([1, W], F32, tag="scr", name=f"scr{n}_{h}")
                        nc.vector.reciprocal_approx_accurate(
                            out=rcrow[:], in_=dnrow[:], scratch=scr[:]
                        )
                        rb = dn_p.tile([DH, W], F32, tag="rb", name=f"rb{n}_{h}")
                        nc.gpsimd.partition_broadcast(rb[:], rcrow[:], channels=DH)
                        if hh == 0:
                            nc.vector.tensor_mul(
                                CT[j][0:DH, W * n : W * (n + 1)], cps[0:DH, :], rb[:]
                            )
                        else:
                            stg = dn_p.tile([DH, W], F32R, tag="stg", name=f"stg{n}_{h}")
                            nc.vector.tensor_mul(stg[:], cps[0:DH, :], rb[:])
                            nc.sync.dma_start(
                                out=CT[j][DH:P, W * n : W * (n + 1)], in_=stg[:]
                            )

        # ---------------- phase 3: output projection ------------------
        with (
            tc.tile_pool(name="wo", bufs=1) as wo_p,
            tc.tile_pool(name="ob", bufs=3) as ob_p,
            tc.tile_pool(name="ps3", bufs=4, space="PSUM") as ps3,
        ):
            WO = [wo_p.tile([P, D], F32R, tag=f"wo{i}", name=f"wo{i}") for i in range(DJ)]
            for i in range(DJ):
                nc.sync.dma_start(out=WO[i][:], in_=woT[P * i : P * (i + 1), :])
            for j in range(DJ):
                for sp in range(SLOC // 512):
                    ps = ps3.tile([P, 512], F32, tag="ps3")
                    for i in range(DJ):
                        nc.tensor.matmul(
                            ps[:],
                            WO[i][:, P * j : P * (j + 1)],
                            CT[i][:, 512 * sp : 512 * (sp + 1)],
                            start=(i == 0),
                            stop=(i == DJ - 1),
                        )
                    osb = ob_p.tile([P, 512], F32, tag="ob")
                    nc.scalar.activation(
                        osb[:], ps[:], AF.Identity, bias=boe_sb[:, j : j + 1]
                    )
                    nc.sync.dma_start(
                        out=outT[P * j : P * (j + 1), 512 * sp : 512 * (sp + 1)],
                        in_=osb[:],
                    )

    nc.compile()
    return nc


def _host_prep(hidden_states, attention_mask, Wq, bq, Wk, bk, Wv, bv, Wo, bo):
    """Build per-core input maps. Returns (in_maps, general_mask)."""
    hs = np.asarray(hidden_states, dtype=np.float32)
    am = np.asarray(attention_mask, dtype=np.float32)
    Wq = np.asarray(Wq, dtype=np.float32)
    Wk = np.asarray(Wk, dtype=np.float32)
    Wv = np.asarray(Wv, dtype=np.float32)
    Wo = np.asarray(Wo, dtype=np.float32)
    bq = np.asarray(bq, dtype=np.float32)
    bv = np.asarray(bv, dtype=np.float32)
    bo = np.asarray(bo, dtype=np.float32)

    general = bool(np.any(am != 0.0))
    NS = 3 if general else 2
    scale = 1.0 / np.sqrt(np.float32(DH))

    wqT = np.ascontiguousarray(Wq.T * scale)
    wkT = np.ascontiguousarray(Wk.T)
    wvT = np.ascontiguousarray(Wv.T)
    woT = np.ascontiguousarray(Wo.T)
    bq_s = (bq * scale).astype(np.float32)
    bo_eff = (bo + Wo @ bv).astype(np.float32)

    # band validity per (tile t, partition p, q): kpos_w = 128 t + p
    t_idx = np.arange(NT)[:, None, None]
    p_idx = np.arange(P)[None, :, None]
    q_idx = np.arange(W)[None, None, :]
    kpos_w = P * t_idx + p_idx                      # [6,128,1]
    band_ok = np.abs(kpos_w - W - q_idx) <= W       # [6,128,256]

    in_maps = []
    for c in range(NCORES):
        bi, g = divmod(c, G)
        lo = SLOC * g - W
        halo = np.zeros((SH, D), dtype=np.float32)
        s0, s1 = max(lo, 0), min(lo + SH, S)
        halo[s0 - lo : s1 - lo] = hs[bi, s0:s1]
        hsT_c = np.ascontiguousarray(halo.T)

        m = np.empty((NB, NS, P, 512), dtype=np.float32)
        slot_tiles = [(0, 1), (2, 3), (4, 5)] if general else [(0, 1), (4, 5)]
        for n in range(NB):
            gc = NB * g + n                          # global chunk index
            kglob = W * gc + kpos_w - W              # [6,128,1]
            inb = (kglob >= 0) & (kglob < S)
            if general:
                kb = np.where(
                    inb, -am[bi, np.clip(kglob, 0, S - 1)], 0.0
                )                                    # [6,128,1] key bias
            else:
                kb = np.zeros_like(kglob, dtype=np.float32)
            valid = band_ok & inb
            mt = np.where(valid, kb, NEG).astype(np.float32)  # [6,128,256]
            for sl, (ta, tb) in enumerate(slot_tiles):
                m[n, sl, :, 0:256] = mt[ta]
                m[n, sl, :, 256:512] = mt[tb]

        in_maps.append(
            {
                "hsT": hsT_c,
                "wqT": wqT,
                "wkT": wkT,
                "wvT": wvT,
                "woT": woT,
                "bq": bq_s,
                "boe": bo_eff,
                "masks": m,
            }
        )
    return in_maps, general


def _run(inputs: dict, trace: bool = False):
    """Run the sharded kernel. Returns (full_output, BassKernelResults)."""
    from concourse.bass_utils import run_bass_kernel_spmd

    in_maps, general = _host_prep(**inputs)
    key = ("nc", general)
    if key not in _PROGRAM_CACHE:
        _PROGRAM_CACHE[key] = _build_program(general)
    nc = _PROGRAM_CACHE[key]

    res = run_bass_kernel_spmd(
        nc, in_maps, list(range(NCORES)), trace=trace
    )
    out = np.empty((B, S, D), dtype=np.float32)
    for c in range(NCORES):
        bi, g = divmod(c, G)
        out[bi, SLOC * g : SLOC * (g + 1), :] = res.results[c]["outT"].T
    return out, res


def kernel(**inputs) -> np.ndarray:
    out, _ = _run(inputs, trace=False)
    return out



# revision 4
# speedup vs baseline: 1.7008x; 1.7008x over previous
"""Longformer sliding-window self-attention (MBart variant) on 8 TRN2 cores.

Sequence-parallel sharding: core c -> batch c//4, queries
[1024*(c%4), 1024*(c%4+1)). Each core gets a halo'd, transposed,
bf16-cast hidden-state slice (1536 rows), computes Q/K/V projections,
banded attention (4 chunks x 256 q x 768-key windows), and the output
projection, returning a [768, 1024] f32 slice.

Fast path (attention_mask == 0), all matmuls bf16:
  - query scale 1/sqrt(64) folded into Wq/bq on host; bk drops out of
    softmax; bv folded into bo_eff = bo + Wo @ bv.
  - sequence-edge key masking is data-driven: V rows of out-of-range
    keys are zero (zero-padded halo), and each head's stationary VA
    block is [valid01*ones(64) | v(64)] so the softmax denominator
    (rows 0-63 of the ctx psum, replicated by the matmul) excludes
    out-of-range keys. No per-core mask tensors.
  - the band mask reduces to two constant 128x128 triangles applied to
    4 half-tiles per (chunk, head) via 2 strided DVE adds; the two
    always-out-of-band half-tiles are never computed/read.
  - softmax normalize: DVE reciprocal of the replicated denominator
    rows + one psum*sbuf multiply per (pair, head) over 512 queries.
  - score matmuls for the two heads of a Q/K tile go to PE row groups
    (0,0)/(64,0) and run concurrently in the array.

General attention_mask path: the original fp32r kernel (slow, correct).
"""

import numpy as np

B, S, D, H = 2, 4096, 768, 12
DH = D // H            # 64
W = 256                # one-sided window == chunk size b
NEG = -1e9
NCORES = 8
G = 4                  # sequence groups per batch
SLOC = S // G          # 1024 queries per core
SH = SLOC + 2 * W      # 1536 halo'd rows per core
NB = SLOC // W         # 4 chunks per core
NT = 3 * W // 128      # 6 key tiles of 128 per chunk window
P = 128
DJ = D // P            # 6 tiles of 128 over the model dim
NST = SH // P          # 12 halo seq tiles

_PROGRAM_CACHE: dict = {}

# hh=1 normalize writes CT[j][64:128] directly (DVE out at partition
# base 64). Set False to route through a staging tile + DMA instead.
_HH1_DIRECT = True


def _build_fast():
    import concourse.bacc as bacc
    import concourse.mybir as mybir
    import concourse.tile as tile
    from contextlib import ExitStack

    F32 = mybir.dt.float32
    BF16 = mybir.dt.bfloat16
    AF = mybir.ActivationFunctionType

    nc = bacc.Bacc("TRN2", target_bir_lowering=False, debug=False)

    hsT = nc.dram_tensor("hsT", [D, SH], BF16, kind="ExternalInput")
    wqT = nc.dram_tensor("wqT", [D, D], BF16, kind="ExternalInput")
    wkT = nc.dram_tensor("wkT", [D, D], BF16, kind="ExternalInput")
    wvT = nc.dram_tensor("wvT", [D, D], BF16, kind="ExternalInput")
    woT = nc.dram_tensor("woT", [D, D], BF16, kind="ExternalInput")
    bq = nc.dram_tensor("bq", [P, DJ], F32, kind="ExternalInput")
    boe = nc.dram_tensor("boe", [P, DJ], F32, kind="ExternalInput")
    validf = nc.dram_tensor("validf", [NST, P, D], BF16, kind="ExternalInput")
    bandA = nc.dram_tensor("bandA", [P, 2 * P], F32, kind="ExternalInput")
    bandB = nc.dram_tensor("bandB", [P, 2 * P], F32, kind="ExternalInput")
    outT = nc.dram_tensor("outT", [D, SLOC], F32, kind="ExternalOutput")

    with tile.TileContext(nc) as tc, ExitStack() as stack:
        const = stack.enter_context(tc.tile_pool(name="const", bufs=1))
        big = stack.enter_context(tc.tile_pool(name="big", bufs=1))
        wts = stack.enter_context(tc.tile_pool(name="wts", bufs=1))

        HS = [big.tile([P, SH], BF16, tag=f"hs{i}", name=f"hs{i}") for i in range(DJ)]
        for i in range(DJ):
            nc.sync.dma_start(out=HS[i][:], in_=hsT[P * i : P * (i + 1), :])

        WV = [wts.tile([P, D], BF16, tag=f"wv{i}", name=f"wv{i}") for i in range(DJ)]
        WK = [wts.tile([P, D], BF16, tag=f"wk{i}", name=f"wk{i}") for i in range(DJ)]
        WQ = [wts.tile([P, D], BF16, tag=f"wq{i}", name=f"wq{i}") for i in range(DJ)]
        WO = [wts.tile([P, D], BF16, tag=f"wo{i}", name=f"wo{i}") for i in range(DJ)]
        for i in range(DJ):
            nc.sync.dma_start(out=WV[i][:], in_=wvT[P * i : P * (i + 1), :])
        for i in range(DJ):
            nc.sync.dma_start(out=WK[i][:], in_=wkT[P * i : P * (i + 1), :])
        for i in range(DJ):
            nc.sync.dma_start(out=WQ[i][:], in_=wqT[P * i : P * (i + 1), :])

        bq_sb = const.tile([P, DJ], F32, tag="bq")
        nc.sync.dma_start(out=bq_sb[:], in_=bq[:, :])
        boe_sb = const.tile([P, DJ], F32, tag="boe")
        nc.sync.dma_start(out=boe_sb[:], in_=boe[:, :])
        mA = const.tile([P, 2 * P], F32, tag="mA")
        nc.sync.dma_start(out=mA[:], in_=bandA[:, :])
        mB = const.tile([P, 2 * P], F32, tag="mB")
        nc.sync.dma_start(out=mB[:], in_=bandB[:, :])

        QT = [big.tile([P, SLOC], BF16, tag=f"qt{j}", name=f"qt{j}") for j in range(DJ)]
        KT = [big.tile([P, SH], BF16, tag=f"kt{j}", name=f"kt{j}") for j in range(DJ)]
        CT = [big.tile([P, SLOC], BF16, tag=f"ct{j}", name=f"ct{j}") for j in range(DJ)]
        # VA[st]: per head h, cols [128h, 128h+64) = valid01 (denominator
        # ones block), cols [128h+64, 128h+128) = V.
        VA = [big.tile([P, H * P], BF16, tag=f"va{s}", name=f"va{s}") for s in range(NST)]
        for st in range(NST):
            vv = VA[st].rearrange("p (h x) -> p h x", x=P)
            nc.sync.dma_start(
                out=vv[:, :, 0:DH],
                in_=validf[st].rearrange("p (h x) -> p h x", x=DH),
            )

        for i in range(DJ):
            nc.sync.dma_start(out=WO[i][:], in_=woT[P * i : P * (i + 1), :])

        # ---------------- phase 1: projections ------------------------
        with tc.tile_pool(name="ps1", bufs=4, space="PSUM") as ps1:
            # V = hs @ Wv.T : per seq tile, [128 seq, 768 head-dims]
            for st in range(NST):
                for half in range(2):
                    ps = ps1.tile([P, 512], F32, tag="ps1")
                    for i in range(DJ):
                        nc.tensor.matmul(
                            ps[:, 0:384],
                            HS[i][:, P * st : P * (st + 1)],
                            WV[i][:, 384 * half : 384 * (half + 1)],
                            start=(i == 0),
                            stop=(i == DJ - 1),
                        )
                    vv = VA[st].rearrange("p (h x) -> p h x", x=P)
                    nc.vector.tensor_copy(
                        vv[:, 6 * half : 6 * (half + 1), DH:P],
                        ps[:, 0:384].rearrange("p (h e) -> p h e", e=DH),
                    )

            # KT[j] = Wk @ hs_halo.T
            for sp in range(SH // 512):
                for j in range(DJ):
                    ps = ps1.tile([P, 512], F32, tag="ps1")
                    for i in range(DJ):
                        nc.tensor.matmul(
                            ps[:],
                            WK[i][:, P * j : P * (j + 1)],
                            HS[i][:, 512 * sp : 512 * (sp + 1)],
                            start=(i == 0),
                            stop=(i == DJ - 1),
                        )
                    nc.scalar.activation(
                        KT[j][:, 512 * sp : 512 * (sp + 1)], ps[:], AF.Copy
                    )

            # QT[j] = (Wq/8) @ hs_loc.T + bq/8
            for sp in range(SLOC // 512):
                for j in range(DJ):
                    ps = ps1.tile([P, 512], F32, tag="ps1")
                    for i in range(DJ):
                        nc.tensor.matmul(
                            ps[:],
                            WQ[i][:, P * j : P * (j + 1)],
                            HS[i][:, W + 512 * sp : W + 512 * (sp + 1)],
                            start=(i == 0),
                            stop=(i == DJ - 1),
                        )
                    nc.scalar.activation(
                        QT[j][:, 512 * sp : 512 * (sp + 1)],
                        ps[:],
                        AF.Identity,
                        bias=bq_sb[:, j : j + 1],
                    )

        # ---------------- phase 2: banded attention -------------------
        with (
            tc.tile_pool(name="exp", bufs=1) as exp_p,
            tc.tile_pool(name="rcb", bufs=2) as rc_p,
            tc.tile_pool(name="stg", bufs=2) as stg_p,
            tc.tile_pool(name="pss", bufs=1, space="PSUM") as pss,
            tc.tile_pool(name="psc", bufs=1, space="PSUM") as psc,
        ):
            for p_ in range(NB // 2):
                for j in range(DJ):
                    EX = {}
                    for c in range(2):
                        n = 2 * p_ + c
                        sps = [
                            pss.tile([P, NT * W], F32, tag=f"s{hh}", name=f"s{n}_{j}_{hh}")
                            for hh in range(2)
                        ]
                        # banded scores, two heads concurrent via PE row tiles
                        for t in range(NT):
                            for hh in range(2):
                                r0 = DH * hh
                                kc = W * n + P * t
                                if t == 0:
                                    oc, qc, qn = 0, W * n, P
                                elif t == NT - 1:
                                    oc, qc, qn = NT * W - P, W * n + P, P
                                else:
                                    oc, qc, qn = W * t, W * n, W
                                nc.tensor.matmul(
                                    sps[hh][:, oc : oc + qn],
                                    KT[j][r0 : r0 + DH, kc : kc + P],
                                    QT[j][r0 : r0 + DH, qc : qc + qn],
                                    start=True,
                                    stop=True,
                                )
                        for hh in range(2):
                            v = sps[hh].rearrange("p (b s) -> p b s", s=384)
                            mAv = mA[:].rearrange("p (b s) -> p b s", s=P)
                            mBv = mB[:].rearrange("p (b s) -> p b s", s=P)
                            nc.vector.tensor_add(
                                v[:, 0:2, 0:P], v[:, 0:2, 0:P], mAv
                            )
                            nc.vector.tensor_add(
                                v[:, 2:4, 2 * P : 3 * P], v[:, 2:4, 2 * P : 3 * P], mBv
                            )
                            ex = exp_p.tile(
                                [P, NT * W], BF16, tag=f"e{c}{hh}",
                                name=f"e{n}_{j}_{hh}", bufs=2,
                            )
                            nc.scalar.activation(ex[:], sps[hh][:], AF.Exp)
                            EX[(c, hh)] = ex

                    # ctx + replicated denominators: cps rows 0-63 denom,
                    # 64-127 ctx, cols = 512 queries of the chunk pair
                    cps = [
                        psc.tile([P, 512], F32, tag=f"c{hh}", name=f"c{p_}_{j}_{hh}")
                        for hh in range(2)
                    ]
                    for hh in range(2):
                        h = 2 * j + hh
                        for c in range(2):
                            n = 2 * p_ + c
                            for t in range(NT):
                                if t == 0:
                                    oc, ec, qn = W * c, 0, P
                                elif t == NT - 1:
                                    oc, ec, qn = W * c + P, NT * W - P, P
                                else:
                                    oc, ec, qn = W * c, W * t, W
                                nc.tensor.matmul(
                                    cps[hh][:, oc : oc + qn],
                                    VA[2 * n + t][:, P * h : P * (h + 1)],
                                    EX[(c, hh)][:, ec : ec + qn],
                                    start=(c == 0 and t == 0),
                                    stop=(c == 1 and t == NT - 1),
                                    skip_group_check=True,
                                )
                    for hh in range(2):
                        rcb = rc_p.tile([DH, 512], F32, tag=f"r{hh}", name=f"r{p_}_{j}_{hh}")
                        nc.vector.reciprocal_approx_fast(
                            out=rcb[:], in_=cps[hh][0:DH, :]
                        )
                        qlo = 512 * p_
                        if hh == 0:
                            nc.vector.tensor_mul(
                                CT[j][0:DH, qlo : qlo + 512],
                                cps[hh][DH:P, :],
                                rcb[:],
                            )
                        elif _HH1_DIRECT:
                            nc.vector.tensor_mul(
                                CT[j][DH:P, qlo : qlo + 512],
                                cps[hh][DH:P, :],
                                rcb[:],
                            )
                        else:
                            stg = stg_p.tile([DH, 512], BF16, tag="stg")
                            nc.vector.tensor_mul(stg[:], cps[hh][DH:P, :], rcb[:])
                            nc.sync.dma_start(
                                out=CT[j][DH:P, qlo : qlo + 512], in_=stg[:]
                            )

        # ---------------- phase 3: output projection ------------------
        with (
            tc.tile_pool(name="ob", bufs=3) as ob_p,
            tc.tile_pool(name="ps3", bufs=4, space="PSUM") as ps3,
        ):
            for j in range(DJ):
                for sp in range(SLOC // 512):
                    ps = ps3.tile([P, 512], F32, tag="ps3")
                    for i in range(DJ):
                        nc.tensor.matmul(
                            ps[:],
                            WO[i][:, P * j : P * (j + 1)],
                            CT[i][:, 512 * sp : 512 * (sp + 1)],
                            start=(i == 0),
                            stop=(i == DJ - 1),
                        )
                    osb = ob_p.tile([P, 512], F32, tag="ob")
                    nc.scalar.activation(
                        osb[:], ps[:], AF.Identity, bias=boe_sb[:, j : j + 1]
                    )
                    nc.sync.dma_start(
                        out=outT[P * j : P * (j + 1), 512 * sp : 512 * (sp + 1)],
                        in_=osb[:],
                    )

    nc.compile()
    return nc


def _host_prep_fast(hidden_states, attention_mask, Wq, bq, Wk, bk, Wv, bv, Wo, bo):
    import ml_dtypes

    BF = ml_dtypes.bfloat16
    hs = np.asarray(hidden_states, dtype=np.float32)
    Wq = np.asarray(Wq, dtype=np.float32)
    Wk = np.asarray(Wk, dtype=np.float32)
    Wv = np.asarray(Wv, dtype=np.float32)
    Wo = np.asarray(Wo, dtype=np.float32)
    bq = np.asarray(bq, dtype=np.float32)
    bv = np.asarray(bv, dtype=np.float32)
    bo = np.asarray(bo, dtype=np.float32)

    scale = 1.0 / np.sqrt(np.float32(DH))
    wqT = np.ascontiguousarray(Wq.T * scale).astype(BF)
    wkT = np.ascontiguousarray(Wk.T).astype(BF)
    wvT = np.ascontiguousarray(Wv.T).astype(BF)
    woT = np.ascontiguousarray(Wo.T).astype(BF)
    bq_s = (bq * scale).reshape(DJ, P).T.copy()          # [128, 6] f32
    bo_eff = (bo + Wo @ bv).reshape(DJ, P).T.copy()      # [128, 6] f32

    # band triangle constants (identical for every core/chunk)
    pp = np.arange(P)[:, None]
    qq = np.arange(P)[None, :]
    tri_ge = np.where(pp >= qq, 0.0, NEG).astype(np.float32)   # t0/t1 edges
    tri_le = np.where(pp <= qq, 0.0, NEG).astype(np.float32)   # t4/t5 edges
    bandA = np.concatenate([tri_ge, tri_ge], axis=1)
    bandB = np.concatenate([tri_le, tri_le], axis=1)

    in_maps = []
    for c in range(NCORES):
        bi, g = divmod(c, G)
        lo = SLOC * g - W
        halo = np.zeros((SH, D), dtype=np.float32)
        s0, s1 = max(lo, 0), min(lo + SH, S)
        halo[s0 - lo : s1 - lo] = hs[bi, s0:s1]
        hsT_c = np.ascontiguousarray(halo.T).astype(BF)

        kglob = lo + np.arange(SH)
        valid = ((kglob >= 0) & (kglob < S)).astype(np.float32)   # [1536]
        validf = np.broadcast_to(
            valid.reshape(NST, P, 1, 1), (NST, P, H, DH)
        ).reshape(NST, P, D).astype(BF)

        in_maps.append(
            {
                "hsT": hsT_c,
                "wqT": wqT,
                "wkT": wkT,
                "wvT": wvT,
                "woT": woT,
                "bq": bq_s,
                "boe": bo_eff,
                "validf": np.ascontiguousarray(validf),
                "bandA": bandA,
                "bandB": bandB,
            }
        )
    return in_maps


# ---------------------------------------------------------------------------
# general attention_mask fallback: original fp32r kernel
# ---------------------------------------------------------------------------

def _build_general(general_mask: bool = True):
    import concourse.bacc as bacc
    import concourse.mybir as mybir
    import concourse.tile as tile
    from contextlib import ExitStack

    F32 = mybir.dt.float32
    F32R = mybir.dt.float32r
    AF = mybir.ActivationFunctionType
    NS = 3 if general_mask else 2
    MCOLS = NB * NS * 512

    nc = bacc.Bacc("TRN2", target_bir_lowering=False, debug=False)

    hsT = nc.dram_tensor("hsT", [D, SH], F32R, kind="ExternalInput")
    wqT = nc.dram_tensor("wqT", [D, D], F32R, kind="ExternalInput")
    wkT = nc.dram_tensor("wkT", [D, D], F32R, kind="ExternalInput")
    wvT = nc.dram_tensor("wvT", [D, D], F32R, kind="ExternalInput")
    woT = nc.dram_tensor("woT", [D, D], F32R, kind="ExternalInput")
    bq = nc.dram_tensor("bq", [D], F32, kind="ExternalInput")
    boe = nc.dram_tensor("boe", [D], F32, kind="ExternalInput")
    masks = nc.dram_tensor("masks", [NB, NS, P, 512], F32, kind="ExternalInput")
    outT = nc.dram_tensor("outT", [D, SLOC], F32, kind="ExternalOutput")

    with tile.TileContext(nc) as tc, ExitStack() as stack:
        const = stack.enter_context(tc.tile_pool(name="const", bufs=1))
        qt_p = stack.enter_context(tc.tile_pool(name="qt", bufs=1))
        kt_p = stack.enter_context(tc.tile_pool(name="kt", bufs=1))
        ct_p = stack.enter_context(tc.tile_pool(name="ct", bufs=1))

        bq_sb = const.tile([P, DJ], F32, tag="bq")
        nc.sync.dma_start(out=bq_sb[:], in_=bq.rearrange("(t p) -> p t", p=P))
        boe_sb = const.tile([P, DJ], F32, tag="boe")
        nc.sync.dma_start(out=boe_sb[:], in_=boe.rearrange("(t p) -> p t", p=P))
        mask_sb = const.tile([P, MCOLS], F32, tag="masks")
        for n in range(NB):
            for sl in range(NS):
                off = (n * NS + sl) * 512
                nc.sync.dma_start(
                    out=mask_sb[:, off : off + 512], in_=masks[n, sl]
                )

        QT = [qt_p.tile([P, SLOC], F32R, tag=f"qt{j}", name=f"qt{j}") for j in range(DJ)]
        KT = [kt_p.tile([P, SH], F32R, tag=f"kt{j}", name=f"kt{j}") for j in range(DJ)]
        CT = [ct_p.tile([P, SLOC], F32R, tag=f"ct{j}", name=f"ct{j}") for j in range(DJ)]

        va_p = stack.enter_context(tc.tile_pool(name="va", bufs=1))
        VA = [va_p.tile([P, H * (DH + 1)], F32R, tag=f"va{s}", name=f"va{s}") for s in range(SH // P)]

        ones_f = const.tile([P, H], F32, tag="ones_f")
        nc.vector.memset(ones_f[:], 1.0)
        for st in range(SH // P):
            view = VA[st].rearrange("p (h e) -> p h e", e=DH + 1)
            nc.vector.tensor_copy(view[:, :, DH : DH + 1], ones_f[:])

        from contextlib import ExitStack as _ES
        hs_stack = _ES()
        hs_p = hs_stack.enter_context(tc.tile_pool(name="hs", bufs=1))
        HS = [hs_p.tile([P, SH], F32R, tag=f"hs{i}", name=f"hs{i}") for i in range(DJ)]
        for i in range(DJ):
            nc.sync.dma_start(out=HS[i][:], in_=hsT[P * i : P * (i + 1), :])

        with (
            tc.tile_pool(name="wv", bufs=1) as wv_p,
            tc.tile_pool(name="ps2", bufs=4, space="PSUM") as ps2,
        ):
            for half in range(2):
                WVt = [wv_p.tile([P, 384], F32R, tag=f"wv{i}", name=f"wv{i}") for i in range(DJ)]
                for i in range(DJ):
                    nc.sync.dma_start(
                        out=WVt[i][:],
                        in_=wvT[P * i : P * (i + 1), 384 * half : 384 * (half + 1)],
                    )
                for st in range(SH // P):
                    ps = ps2.tile([P, 384], F32, tag="ps2")
                    for i in range(DJ):
                        nc.tensor.matmul(
                            ps[:],
                            HS[i][:, P * st : P * (st + 1)],
                            WVt[i][:],
                            start=(i == 0),
                            stop=(i == DJ - 1),
                        )
                    view = VA[st].rearrange("p (h e) -> p h e", e=DH + 1)
                    nc.vector.tensor_copy(
                        view[:, 6 * half : 6 * (half + 1), 0:DH],
                        ps[:].rearrange("p (h e) -> p h e", e=DH),
                    )

        with (
            tc.tile_pool(name="wq", bufs=1) as wq_p,
            tc.tile_pool(name="ps1", bufs=4, space="PSUM") as ps1,
        ):
            for half in range(2):
                WQt = [wq_p.tile([P, 384], F32R, tag=f"wq{i}", name=f"wq{i}") for i in range(DJ)]
                for i in range(DJ):
                    nc.sync.dma_start(
                        out=WQt[i][:],
                        in_=wqT[P * i : P * (i + 1), 384 * half : 384 * (half + 1)],
                    )
                for j in range(3 * half, 3 * half + 3):
                    jc = P * j - 384 * half
                    for sp in range(SLOC // 512):
                        ps = ps1.tile([P, 512], F32, tag="ps1")
                        for i in range(DJ):
                            nc.tensor.matmul(
                                ps[:],
                                WQt[i][:, jc : jc + P],
                                HS[i][:, W + 512 * sp : W + 512 * (sp + 1)],
                                start=(i == 0),
                                stop=(i == DJ - 1),
                            )
                        nc.scalar.activation(
                            QT[j][:, 512 * sp : 512 * (sp + 1)],
                            ps[:],
                            AF.Identity,
                            bias=bq_sb[:, j : j + 1],
                        )

        with (
            tc.tile_pool(name="wk", bufs=1) as wk_p,
            tc.tile_pool(name="ps1b", bufs=4, space="PSUM") as ps1b,
        ):
            for half in range(2):
                WKt = [wk_p.tile([P, 384], F32R, tag=f"wk{i}", name=f"wk{i}") for i in range(DJ)]
                for i in range(DJ):
                    nc.sync.dma_start(
                        out=WKt[i][:],
                        in_=wkT[P * i : P * (i + 1), 384 * half : 384 * (half + 1)],
                    )
                for j in range(3 * half, 3 * half + 3):
                    jc = P * j - 384 * half
                    for sp in range(SH // 512):
                        ps = ps1b.tile([P, 512], F32, tag="ps1b")
                        for i in range(DJ):
                            nc.tensor.matmul(
                                ps[:],
                                WKt[i][:, jc : jc + P],
                                HS[i][:, 512 * sp : 512 * (sp + 1)],
                                start=(i == 0),
                                stop=(i == DJ - 1),
                            )
                        nc.scalar.activation(
                            KT[j][:, 512 * sp : 512 * (sp + 1)], ps[:], AF.Copy
                        )
        hs_stack.close()

        with (
            tc.tile_pool(name="expp", bufs=3) as exp_p,
            tc.tile_pool(name="dn", bufs=4) as dn_p,
            tc.tile_pool(name="pss", bufs=2, space="PSUM") as pss,
            tc.tile_pool(name="psc", bufs=2, space="PSUM") as psc,
        ):
            for n in range(NB):
                for j in range(DJ):
                    sps = [pss.tile([P, NT * W], F32, tag="s", name=f"s{n}_{j}_{k}") for k in range(2)]
                    for t in range(NT):
                        for hh in range(2):
                            r0 = DH * hh
                            nc.tensor.matmul(
                                sps[hh][:, W * t : W * (t + 1)],
                                KT[j][r0 : r0 + DH, W * n + P * t : W * n + P * (t + 1)],
                                QT[j][r0 : r0 + DH, W * n : W * (n + 1)],
                                start=True,
                                stop=True,
                            )
                    for hh in range(2):
                        h = 2 * j + hh
                        moff = n * NS * 512
                        nc.vector.tensor_add(
                            sps[hh][:, 0:512],
                            sps[hh][:, 0:512],
                            mask_sb[:, moff : moff + 512],
                        )
                        nc.vector.tensor_add(
                            sps[hh][:, 512:1024],
                            sps[hh][:, 512:1024],
                            mask_sb[:, moff + 512 : moff + 1024],
                        )
                        lastoff = moff + (NS - 1) * 512
                        nc.vector.tensor_add(
                            sps[hh][:, 1024:1536],
                            sps[hh][:, 1024:1536],
                            mask_sb[:, lastoff : lastoff + 512],
                        )
                        expt = exp_p.tile([P, NT * W], F32R, tag="e", name=f"e{n}_{j}_{hh}")
                        nc.scalar.activation(expt[:], sps[hh][:], AF.Exp)
                        cps = psc.tile([DH + 1, W], F32, tag="c", name=f"c{n}_{h}")
                        for t in range(NT):
                            nc.tensor.matmul(
                                cps[:],
                                VA[2 * n + t][:, (DH + 1) * h : (DH + 1) * (h + 1)],
                                expt[:, W * t : W * (t + 1)],
                                start=(t == 0),
                                stop=(t == NT - 1),
                            )
                        dcp = dn_p.tile([DH + 1, W], F32, tag="dcp", name=f"dcp{n}_{h}")
                        nc.vector.tensor_copy(dcp[DH : DH + 1, :], cps[DH : DH + 1, :])
                        dnrow = dn_p.tile([1, W], F32, tag="dnr", name=f"dnr{n}_{h}")
                        nc.sync.dma_start(out=dnrow[:], in_=dcp[DH : DH + 1, :])
                        rcrow = dn_p.tile([1, W], F32, tag="rcr", name=f"rcr{n}_{h}")
                        scr = dn_p.tile([1, W], F32, tag="scr", name=f"scr{n}_{h}")
                        nc.vector.reciprocal_approx_accurate(
                            out=rcrow[:], in_=dnrow[:], scratch=scr[:]
                        )
                        rb = dn_p.tile([DH, W], F32, tag="rb", name=f"rb{n}_{h}")
                        nc.gpsimd.partition_broadcast(rb[:], rcrow[:], channels=DH)
                        if hh == 0:
                            nc.vector.tensor_mul(
                                CT[j][0:DH, W * n : W * (n + 1)], cps[0:DH, :], rb[:]
                            )
                        else:
                            stg = dn_p.tile([DH, W], F32R, tag="stg", name=f"stg{n}_{h}")
                            nc.vector.tensor_mul(stg[:], cps[0:DH, :], rb[:])
                            nc.sync.dma_start(
                                out=CT[j][DH:P, W * n : W * (n + 1)], in_=stg[:]
                            )

        with (
            tc.tile_pool(name="wo", bufs=1) as wo_p,
            tc.tile_pool(name="ob", bufs=3) as ob_p,
            tc.tile_pool(name="ps3", bufs=4, space="PSUM") as ps3,
        ):
            WOt = [wo_p.tile([P, D], F32R, tag=f"wo{i}", name=f"wo{i}") for i in range(DJ)]
            for i in range(DJ):
                nc.sync.dma_start(out=WOt[i][:], in_=woT[P * i : P * (i + 1), :])
            for j in range(DJ):
                for sp in range(SLOC // 512):
                    ps = ps3.tile([P, 512], F32, tag="ps3")
                    for i in range(DJ):
                        nc.tensor.matmul(
                            ps[:],
                            WOt[i][:, P * j : P * (j + 1)],
                            CT[i][:, 512 * sp : 512 * (sp + 1)],
                            start=(i == 0),
                            stop=(i == DJ - 1),
                        )
                    osb = ob_p.tile([P, 512], F32, tag="ob")
                    nc.scalar.activation(
                        osb[:], ps[:], AF.Identity, bias=boe_sb[:, j : j + 1]
                    )
                    nc.sync.dma_start(
                        out=outT[P * j : P * (j + 1), 512 * sp : 512 * (sp + 1)],
                        in_=osb[:],
                    )

    nc.compile()
    return nc


def _host_prep_general(hidden_states, attention_mask, Wq, bq, Wk, bk, Wv, bv, Wo, bo):
    hs = np.asarray(hidden_states, dtype=np.float32)
    am = np.asarray(attention_mask, dtype=np.float32)
    Wq = np.asarray(Wq, dtype=np.float32)
    Wk = np.asarray(Wk, dtype=np.float32)
    Wv = np.asarray(Wv, dtype=np.float32)
    Wo = np.asarray(Wo, dtype=np.float32)
    bq = np.asarray(bq, dtype=np.float32)
    bv = np.asarray(bv, dtype=np.float32)
    bo = np.asarray(bo, dtype=np.float32)

    NS = 3
    scale = 1.0 / np.sqrt(np.float32(DH))

    wqT = np.ascontiguousarray(Wq.T * scale)
    wkT = np.ascontiguousarray(Wk.T)
    wvT = np.ascontiguousarray(Wv.T)
    woT = np.ascontiguousarray(Wo.T)
    bq_s = (bq * scale).astype(np.float32)
    bo_eff = (bo + Wo @ bv).astype(np.float32)

    t_idx = np.arange(NT)[:, None, None]
    p_idx = np.arange(P)[None, :, None]
    q_idx = np.arange(W)[None, None, :]
    kpos_w = P * t_idx + p_idx
    band_ok = np.abs(kpos_w - W - q_idx) <= W

    in_maps = []
    for c in range(NCORES):
        bi, g = divmod(c, G)
        lo = SLOC * g - W
        halo = np.zeros((SH, D), dtype=np.float32)
        s0, s1 = max(lo, 0), min(lo + SH, S)
        halo[s0 - lo : s1 - lo] = hs[bi, s0:s1]
        hsT_c = np.ascontiguousarray(halo.T)

        m = np.empty((NB, NS, P, 512), dtype=np.float32)
        slot_tiles = [(0, 1), (2, 3), (4, 5)]
        for n in range(NB):
            gc = NB * g + n
            kglob = W * gc + kpos_w - W
            inb = (kglob >= 0) & (kglob < S)
            kb = np.where(inb, -am[bi, np.clip(kglob, 0, S - 1)], 0.0)
            valid = band_ok & inb
            mt = np.where(valid, kb, NEG).astype(np.float32)
            for sl, (ta, tb) in enumerate(slot_tiles):
                m[n, sl, :, 0:256] = mt[ta]
                m[n, sl, :, 256:512] = mt[tb]

        in_maps.append(
            {
                "hsT": hsT_c,
                "wqT": wqT,
                "wkT": wkT,
                "wvT": wvT,
                "woT": woT,
                "bq": bq_s,
                "boe": bo_eff,
                "masks": m,
            }
        )
    return in_maps


def _run(inputs: dict, trace: bool = False):
    """Run the sharded kernel. Returns (full_output, BassKernelResults)."""
    from concourse.bass_utils import run_bass_kernel_spmd

    general = bool(np.any(np.asarray(inputs["attention_mask"]) != 0.0))
    if general:
        in_maps = _host_prep_general(**inputs)
        key = "general"
        builder = _build_general
    else:
        in_maps = _host_prep_fast(**inputs)
        key = "fast"
        builder = _build_fast
    if key not in _PROGRAM_CACHE:
        _PROGRAM_CACHE[key] = builder()
    nc = _PROGRAM_CACHE[key]

    res = run_bass_kernel_spmd(nc, in_maps, list(range(NCORES)), trace=trace)
    out = np.empty((B, S, D), dtype=np.float32)
    for c in range(NCORES):
        bi, g = divmod(c, G)
        out[bi, SLOC * g : SLOC * (g + 1), :] = res.results[c]["outT"].T
    return out, res


def kernel(**inputs) -> np.ndarray:
    out, _ = _run(inputs, trace=False)
    return out
